# revision 22
# baseline (speedup 1.0000x reference)
"""Trainium2 Bass kernel for nn_CrossAttentionLayer (ragged cross-attention + MLP).

v2 design:
- 64 ragged segments -> 8 cores x 8 slots. Segments are assigned to slots by
  a small host-side optimizer so that each slot's compile-time dst width
  D[si] = max(nd) and src block count J[si] = max(ceil(ns/128)) over the 8
  cores sharing that slot are tight (the SPMD program is shared across cores).
- All matmul operands are bf16 (fp32 accumulation in PSUM). fp32r streams at
  ~2 cyc/row and disables fast-weight-load; bf16 runs 1 cyc/row.
- Activations stay channel-major [chan, tok]. V is produced directly in
  natural [tok, chan] layout using srcT as the stationary operand (no PE
  transpose, no band scatter).
- Scores: per src-block j, 4 head matmuls row-tiled (tile_position=(32h,0))
  into one PSUM tile [128, 4, 512]; exp on ScalarE in two 2-head calls with
  the src-pad mask as a per-partition bias.
- msg and den: 4-way column-tiled matmuls (tile_position=(0,32h)) accumulate
  over j; den uses a ones stationary so each head's denominator lands
  replicated across its 32 output partitions.
- 1/den via exp(-ln(den)) on ScalarE: Ln and Exp share one activation table
  set; DVE reciprocal is 8 cyc/elem and ACT Reciprocal is banned.
- The dst residual add and final transpose/unpack happen on the host.
"""
import math
import sys
from contextlib import ExitStack

import numpy as np

try:
    import concourse.bass as bass
except ImportError:
    sys.path.insert(0, "/opt/trn_rl_repo")
    import concourse.bass as bass

import concourse.tile as tile
from concourse import bacc, mybir
from concourse.bass_utils import run_bass_kernel_spmd

# The kernel alternates Exp (softmax) and Ln (reciprocal via exp(-ln(den)))
# activations. bacc's table-load pass picks the first act-table set containing
# each function, which lands Exp in "exp_and_others" and Ln in "natural_log"
# and reloads the ACT tables (~1.3us) on every switch. Both functions coexist
# only in "natural_log_exp_and_others"; hide Exp/Ln from the other sets so the
# pass is forced to pick the shared one. Dict order/size is preserved (set ids
# index into act_info.json).
_orig_get_act_tables = bacc.get_activation_tables


def _patched_get_act_tables(arch):
    tabs = _orig_get_act_tables(arch)
    A = mybir.ActivationFunctionType
    for name, fns in tabs.items():
        if name != "natural_log_exp_and_others":
            fns.discard(A.Exp)
            fns.discard(A.Ln)
    return tabs


bacc.get_activation_tables = _patched_get_act_tables

F32 = mybir.dt.float32
BF16 = mybir.dt.bfloat16

B = 64
LMAX = 512
H = 256          # h_dim
C = 128          # h_div
HEADS = 4
DH = 32
NCORES = 8
SEGS = 8         # segments (slots) per core
NPB = 6          # per-partition bias cols: bq,bk,b1a,b1b,b2a,b2b
MASK_NEG = -30000.0

# weight blob column offsets (bf16 blob)
WQ0, WQ1 = 0, 128
WK0, WK1 = 256, 384
WV0, WV1 = 512, 640
W1_0 = 768           # 3 x 256
W2_0 = 1536          # 2 x 256
ONES32 = 2048        # [128, 32] ones
BVROW = 2080         # row 0: bv (128 cols)
ONESROW = 2208       # row 0: ones (128 cols)
NW16 = 2336


def _to_bf16(x):
    import ml_dtypes
    return np.asarray(x, np.float32).astype(ml_dtypes.bfloat16)


def plan_slots(ns, nd):
    """Assign 64 segments to an 8x8 (core, slot) grid minimizing padded
    attention work. Returns grid[core][slot] = segment id, D[slot], J[slot]."""
    segs = list(range(B))
    jj = [int(math.ceil(max(int(n), 1) / 128.0)) for n in ns]

    def cost_of(order):
        # order: list of 64 seg ids; slot si <- order[8si:8si+8]
        tot = 0.0
        for si in range(SEGS):
            grp = order[8 * si:8 * si + 8]
            Dm = max(int(nd[g]) for g in grp)
            Jm = max(jj[g] for g in grp)
            tot += 4.58 * Jm * Dm + 12.7 * Dm + 560.0 * Jm
        return tot

    cand = []
    cand.append(sorted(segs, key=lambda g: int(nd[g])))
    cand.append(sorted(segs, key=lambda g: (jj[g], int(nd[g]))))
    best = min(cand, key=cost_of)
    best = list(best)
    bc = cost_of(best)
    # local search: swap segments between octiles
    improved = True
    it = 0
    while improved and it < 200:
        improved = False
        it += 1
        for a in range(B):
            for b_ in range(a + 1, B):
                if a // 8 == b_ // 8:
                    continue
                best[a], best[b_] = best[b_], best[a]
                c2 = cost_of(best)
                if c2 < bc - 1e-9:
                    bc = c2
                    improved = True
                else:
                    best[a], best[b_] = best[b_], best[a]
    # build grid: within octile si, order by core index arbitrarily
    grid = [[0] * SEGS for _ in range(NCORES)]
    D = [0] * SEGS
    J = [0] * SEGS
    for si in range(SEGS):
        grp = best[8 * si:8 * si + 8]
        D[si] = -(-max(int(nd[g]) for g in grp) // 4) * 4  # round up to 4
        J[si] = max(jj[g] for g in grp)
        for c_, g in enumerate(grp):
            grid[c_][si] = g
    return grid, D, J


def host_prep(inputs):
    src_h = np.asarray(inputs['src_h'], np.float32)
    dst_h = np.asarray(inputs['dst_h'], np.float32)
    ns = np.asarray(inputs['src_num_verts']).astype(np.int64)
    nd = np.asarray(inputs['dst_num_verts']).astype(np.int64)
    soff = np.concatenate([[0], np.cumsum(ns)[:-1]])
    doff = np.concatenate([[0], np.cumsum(nd)[:-1]])

    grid, D, J = plan_slots(ns, nd)
    SD = sum(D)
    SJ = sum(J)
    SS = 128 * SJ
    doffs = np.concatenate([[0], np.cumsum(D)[:-1]])   # slot dst col offsets
    soffs = np.concatenate([[0], np.cumsum(np.array(J) * 128)[:-1]])
    joffs = np.concatenate([[0], np.cumsum(J)[:-1]])

    # channel permutation: on-chip channel chat = h*DH + d  <-> torch c = d*HEADS + h
    perm = np.empty(C, np.int64)
    for chat in range(C):
        h, d = divmod(chat, DH)
        perm[chat] = d * HEADS + h
    s = 1.0 / math.sqrt(DH)

    f32 = lambda k: np.asarray(inputs[k], np.float32)
    Wq, bq = f32('Wq'), f32('bq')
    Wk, bk = f32('Wk'), f32('bk')
    Wv, bv = f32('Wv'), f32('bv')
    Wm, bm = f32('Wm'), f32('bm')
    W1, b1 = f32('W1'), f32('b1')
    W2, b2 = f32('W2'), f32('b2')
    g1, be1, rm1, rv1 = f32('g1'), f32('be1'), f32('rm1'), f32('rv1')
    g2, be2, rm2, rv2 = f32('g2'), f32('be2'), f32('rm2'), f32('rv2')

    WqT = (Wq[perm] * s).T                      # [256, 128] (chat cols)
    bq_s = bq[perm] * s
    WkT = Wk[perm].T
    bk_r = bk[perm]
    WvT = Wv[perm].T                            # [256 cin, 128 cout(chat)]
    bv_r = bv[perm]
    Wm_p = Wm[:, perm]                          # [128, 128(chat)]
    a1 = g1 / np.sqrt(rv1 + 1e-5)
    W1_f = W1 * a1[:, None]
    b1_f = b1 * a1 + be1 - rm1 * a1
    a2 = g2 / np.sqrt(rv2 + 1e-5)
    W2_f = W2 * a2[:, None]
    b2_f = b2 * a2 + be2 - rm2 * a2
    # fold merge conv (Wm, bm) into W1's msg half
    W1m_p = W1_f[:, H:] @ Wm_p                  # [256, 128(chat)]
    b1_p = b1_f + W1_f[:, H:] @ bm
    W1T = np.concatenate([W1_f[:, :H], W1m_p], axis=1).T  # [384, 256]
    W2T = W2_f.T                                # [256, 256]

    wb16 = np.zeros((128, NW16), np.float32)
    wb16[:, WQ0:WQ0 + 128] = WqT[:128]
    wb16[:, WQ1:WQ1 + 128] = WqT[128:]
    wb16[:, WK0:WK0 + 128] = WkT[:128]
    wb16[:, WK1:WK1 + 128] = WkT[128:]
    wb16[:, WV0:WV0 + 128] = WvT[:128]
    wb16[:, WV1:WV1 + 128] = WvT[128:]
    for kk in range(3):
        wb16[:, W1_0 + kk * 256: W1_0 + (kk + 1) * 256] = W1T[kk * 128:(kk + 1) * 128]
    for kk in range(2):
        wb16[:, W2_0 + kk * 256: W2_0 + (kk + 1) * 256] = W2T[kk * 128:(kk + 1) * 128]
    wb16[:, ONES32:ONES32 + 32] = 1.0
    wb16[0, BVROW:BVROW + 128] = bv_r
    wb16[0, ONESROW:ONESROW + 128] = 1.0
    wb16 = _to_bf16(wb16)

    pbias = np.zeros((128, NPB), np.float32)
    pbias[:, 0] = bq_s
    pbias[:, 1] = bk_r
    pbias[:, 2] = b1_p[:128]
    pbias[:, 3] = b1_p[128:]
    pbias[:, 4] = b2_f[:128]
    pbias[:, 5] = b2_f[128:]

    cores = []
    for c_ in range(NCORES):
        dstT = np.zeros((2, 128, SD), np.float32)
        srcT = np.zeros((2, 128, SS), np.float32)
        maskb = np.full((128, SJ), MASK_NEG, np.float32)
        for si in range(SEGS):
            g = grid[c_][si]
            od, os_, oj = doffs[si], soffs[si], joffs[si]
            dseg = dst_h[doff[g]:doff[g] + nd[g]].T      # [256, nd]
            sseg = src_h[soff[g]:soff[g] + ns[g]].T
            dstT[0, :, od:od + nd[g]] = dseg[:128]
            dstT[1, :, od:od + nd[g]] = dseg[128:]
            srcT[0, :, os_:os_ + ns[g]] = sseg[:128]
            srcT[1, :, os_:os_ + ns[g]] = sseg[128:]
            for j in range(J[si]):
                valid = max(0, min(128, int(ns[g]) - j * 128))
                maskb[:valid, oj + j] = 0.0
        cores.append(dict(dstT=_to_bf16(dstT).reshape(2 * 128, SD),
                          srcT=_to_bf16(srcT).reshape(2 * 128, SS),
                          maskb=maskb))

    shared = dict(wb16=wb16, pbias=pbias)
    meta = dict(nd=nd, doff=doff, grid=grid, D=D, J=J, doffs=doffs,
                soffs=soffs, joffs=joffs, SD=SD, SJ=SJ, SS=SS)
    return cores, shared, meta


def declare_tensors(nc, meta):
    SD, SJ, SS = meta['SD'], meta['SJ'], meta['SS']
    aps = {}
    aps['dstT'] = nc.dram_tensor("dstT", [2 * 128, SD], BF16, kind="ExternalInput").ap()
    aps['srcT'] = nc.dram_tensor("srcT", [2 * 128, SS], BF16, kind="ExternalInput").ap()
    aps['maskb'] = nc.dram_tensor("maskb", [128, SJ], F32, kind="ExternalInput").ap()
    aps['wb16'] = nc.dram_tensor("wb16", [128, NW16], BF16, kind="ExternalInput").ap()
    aps['pbias'] = nc.dram_tensor("pbias", [128, NPB], F32, kind="ExternalInput").ap()
    aps['xout'] = nc.dram_tensor("xout", [2 * 128, SD], BF16, kind="ExternalOutput").ap()
    return aps


def build_body(ctx: ExitStack, tc: tile.TileContext, aps, meta):
    nc = tc.nc
    D, J = meta['D'], meta['J']
    SD, SJ, SS = meta['SD'], meta['SJ'], meta['SS']
    doffs, soffs, joffs = meta['doffs'], meta['soffs'], meta['joffs']

    wp = ctx.enter_context(tc.tile_pool(name="wp", bufs=1))
    inp = ctx.enter_context(tc.tile_pool(name="inp", bufs=1))
    act = ctx.enter_context(tc.tile_pool(name="act", bufs=1))
    eat = ctx.enter_context(tc.tile_pool(name="eat", bufs=1))
    out = ctx.enter_context(tc.tile_pool(name="outp", bufs=1))
    gp = ctx.enter_context(tc.tile_pool(name="gp", bufs=2, space="PSUM"))
    scp = ctx.enter_context(tc.tile_pool(name="scp", bufs=1, space="PSUM"))
    mdp = ctx.enter_context(tc.tile_pool(name="mdp", bufs=2, space="PSUM"))

    # --- inputs/weights, ordered so slot 0's operands land first ---
    dst_t = [inp.tile([128, SD], BF16, tag=f"dst{a}", name=f"dst{a}") for a in range(2)]
    src_t = [inp.tile([128, SS], BF16, tag=f"src{a}", name=f"src{a}") for a in range(2)]
    wb = wp.tile([128, NW16], BF16, tag="wb")
    pb = wp.tile([128, NPB], F32, tag="pb")
    maskb = wp.tile([128, SJ], F32, tag="maskb")
    c0d, c0s = doffs[1], soffs[1]
    g0d, g0s = doffs[4], soffs[4]
    nc.scalar.dma_start(out=wb[:, :W1_0], in_=aps['wb16'][:, :W1_0])
    for a in range(2):
        nc.scalar.dma_start(out=dst_t[a][:, :c0d], in_=aps['dstT'][a * 128:(a + 1) * 128, :c0d])
        nc.sync.dma_start(out=src_t[a][:, :c0s], in_=aps['srcT'][a * 128:(a + 1) * 128, :c0s])
    nc.sync.dma_start(out=pb[:], in_=aps['pbias'][:])
    nc.scalar.dma_start(out=maskb[:], in_=aps['maskb'][:])
    nc.sync.dma_start(out=wb[:, W1_0:], in_=aps['wb16'][:, W1_0:])
    for a in range(2):
        nc.sync.dma_start(out=dst_t[a][:, c0d:g0d], in_=aps['dstT'][a * 128:(a + 1) * 128, c0d:g0d])
        nc.sync.dma_start(out=src_t[a][:, c0s:g0s], in_=aps['srcT'][a * 128:(a + 1) * 128, c0s:g0s])
    for a in range(2):
        nc.sync.dma_start(out=dst_t[a][:, g0d:], in_=aps['dstT'][a * 128:(a + 1) * 128, g0d:])
        nc.sync.dma_start(out=src_t[a][:, g0s:], in_=aps['srcT'][a * 128:(a + 1) * 128, g0s:])

    xout_t = [out.tile([128, SD], BF16, tag=f"xo{o}", name=f"xo{o}") for o in range(2)]

    msgn_ts = {}
    # whole-core projection outputs (produced slot 0 first, rest batched)
    q_t = act.tile([128, SD], BF16, tag="q", name="q_all")
    k_t = act.tile([128, SS], BF16, tag="k", name="k_all")
    v_nat = act.tile([128, SJ, 128], BF16, tag="v", name="v_all")

    def emit_q_chunk(lo, hi):
        ps_q = gp.tile([128, 512], F32, tag="gp", name=f"psq{lo}")
        for a in range(2):
            wq_a = (WQ0, WQ1)[a]
            nc.tensor.matmul(ps_q[:, :hi - lo], wb[:, wq_a:wq_a + 128],
                             dst_t[a][:, lo:hi], start=(a == 0), stop=(a == 1))
        nc.vector.tensor_scalar_add(q_t[:, lo:hi], ps_q[:, :hi - lo], pb[:, 0:1])

    def emit_k_chunk(lo, hi):
        ps_k = gp.tile([128, 512], F32, tag="gp", name=f"psk{lo}")
        for a in range(2):
            nc.tensor.matmul(ps_k[:, :hi - lo], wb[:, (WK0, WK1)[a]:(WK0, WK1)[a] + 128],
                             src_t[a][:, lo:hi], start=(a == 0), stop=(a == 1))
        nc.vector.tensor_scalar_add(k_t[:, lo:hi], ps_k[:, :hi - lo], pb[:, 1:2])

    def emit_v_slot(si):
        # V in natural [tok, chan] layout: lhsT = srcT block, rhs = WvT half
        Js = J[si]
        os_, oj = soffs[si], joffs[si]
        ps_v = gp.tile([128, 4, 128], F32, tag="gp", name=f"psv{si}")
        for j in range(Js):
            for a in range(2):
                nc.tensor.matmul(ps_v[:, j, :], src_t[a][:, os_ + 128 * j: os_ + 128 * (j + 1)],
                                 wb[:, (WV0, WV1)[a]:(WV0, WV1)[a] + 128],
                                 start=(a == 0), stop=False)
            nc.tensor.matmul(ps_v[:, j, :], wb[0:1, ONESROW:ONESROW + 128],
                             wb[0:1, BVROW:BVROW + 128], start=False, stop=True)
        nc.vector.tensor_copy(v_nat[:, oj:oj + Js, :], ps_v[:, :Js, :])

    def chunks(lo, hi, step=512):
        return [(a, min(a + step, hi)) for a in range(lo, hi, step)]

    md_ps = {}
    e_tiles = {}

    def emit_sc_exp(si, j):
        Ds = D[si]
        od, os_, oj = doffs[si], soffs[si], joffs[si]
        if j == 0:
            md_ps[si] = (mdp.tile([128, 512], F32, tag="md", name=f"psmsg{si}"),
                         mdp.tile([128, 512], F32, tag="md", name=f"psden{si}"))
        # the scores->exp chain feeds the bottleneck engine (ScalarE): give it
        # top scheduling priority so mlp/proj matmuls never delay it
        with tc.high_priority():
            ps_sc = scp.tile([128, 4, 512], F32, tag="sc", name=f"pssc{si}_{j}")
            for h in range(HEADS):
                nc.tensor.matmul(ps_sc[:, h, :Ds],
                                 k_t[32 * h:32 * h + 32, os_ + 128 * j:os_ + 128 * (j + 1)],
                                 q_t[32 * h:32 * h + 32, od:od + Ds],
                                 start=True, stop=True, tile_position=(32 * h, 0))
            e_t = eat.tile([128, 4, 512], BF16, tag="E", name=f"E{si}_{j}", bufs=3)
            for hp in range(2):
                nc.scalar.activation(e_t[:, 2 * hp:2 * hp + 2, :Ds],
                                     ps_sc[:, 2 * hp:2 * hp + 2, :Ds],
                                     mybir.ActivationFunctionType.Exp,
                                     bias=maskb[:, oj + j: oj + j + 1])
        e_tiles[(si, j)] = e_t

    def emit_md(si, j):
        Ds, Js = D[si], J[si]
        oj = joffs[si]
        ps_msg, ps_den = md_ps[si]
        e_t = e_tiles.pop((si, j))
        for h in range(HEADS):
            nc.tensor.matmul(ps_msg[32 * h:32 * h + 32, :Ds],
                             v_nat[:, oj + j, 32 * h:32 * h + 32],
                             e_t[:, h, :Ds],
                             start=(j == 0), stop=(j == Js - 1),
                             tile_position=(0, 32 * h),
                             skip_group_check=True)
        for h in range(HEADS):
            nc.tensor.matmul(ps_den[32 * h:32 * h + 32, :Ds],
                             wb[:, ONES32:ONES32 + 32],
                             e_t[:, h, :Ds],
                             start=(j == 0), stop=(j == Js - 1),
                             tile_position=(0, 32 * h),
                             skip_group_check=True)

    def emit_tail(si):
        Ds = D[si]
        ps_msg, ps_den = md_ps.pop(si)
        ln_d = eat.tile([128, 512], F32, tag="lnd", name=f"lnd{si}", bufs=2)
        nc.scalar.activation(ln_d[:, :Ds], ps_den[:, :Ds],
                             mybir.ActivationFunctionType.Ln)
        r_t = eat.tile([128, 512], F32, tag="r", name=f"r{si}", bufs=2)
        nc.scalar.activation(r_t[:, :Ds], ln_d[:, :Ds],
                             mybir.ActivationFunctionType.Exp, scale=-1.0)
        msgn = act.tile([128, 512], BF16, tag="msgn", name=f"msgn{si}", bufs=2)
        nc.vector.tensor_mul(msgn[:, :Ds], ps_msg[:, :Ds], r_t[:, :Ds])
        msgn_ts[si] = msgn

    def emit_mlp(si):
        Ds = D[si]
        od = doffs[si]
        msgn = msgn_ts[si]
        y1 = [None, None]
        for o in range(2):
            ps_y = gp.tile([128, 512], F32, tag="gp", name=f"psy{si}_{o}")
            rhs = [dst_t[0][:, od:od + Ds], dst_t[1][:, od:od + Ds], msgn[:, :Ds]]
            for kk in range(3):
                nc.tensor.matmul(ps_y[:, :Ds],
                                 wb[:, W1_0 + 256 * kk + 128 * o: W1_0 + 256 * kk + 128 * o + 128],
                                 rhs[kk], start=(kk == 0), stop=(kk == 2))
            y1_t = act.tile([128, 512], BF16, tag=f"y1_{o}", name=f"y1_{si}_{o}", bufs=2)
            nc.vector.tensor_scalar(y1_t[:, :Ds], ps_y[:, :Ds], pb[:, 2 + o:3 + o], 0.0,
                                    op0=mybir.AluOpType.add, op1=mybir.AluOpType.max)
            y1[o] = y1_t
        for o in range(2):
            ps_z = gp.tile([128, 512], F32, tag="gp", name=f"psz{si}_{o}")
            for kk in range(2):
                nc.tensor.matmul(ps_z[:, :Ds],
                                 wb[:, W2_0 + 256 * kk + 128 * o: W2_0 + 256 * kk + 128 * o + 128],
                                 y1[kk][:, :Ds], start=(kk == 0), stop=(kk == 1))
            nc.vector.tensor_scalar_add(xout_t[o][:, od:od + Ds], ps_z[:, :Ds],
                                        pb[:, 4 + o:5 + o])

        # write back per slot
        for o in range(2):
            nc.sync.dma_start(out=aps['xout'][o * 128:(o + 1) * 128, od:od + Ds],
                              in_=xout_t[o][:, od:od + Ds])

    # explicit software pipeline over the flat (slot, j) stream: the scores/exp
    # stream runs one step ahead of the msg/den stream (so the ScalarE exp chain
    # is never starved by msg/den or mlp work), with slot-granular projections
    # two slots ahead and mlp trailing each slot's tail.
    def emit_proj(si):
        emit_q_chunk(doffs[si], doffs[si] + D[si])
        emit_k_chunk(soffs[si], soffs[si] + 128 * J[si])
        emit_v_slot(si)

    emit_proj(0)
    emit_proj(1)
    steps = [(si, j) for si in range(SEGS) for j in range(J[si])]
    prev = None
    for step in steps:
        emit_sc_exp(*step)
        if prev is not None:
            emit_md(*prev)
            if prev[1] == J[prev[0]] - 1:
                s0 = prev[0]
                emit_tail(s0)
                if s0 + 2 < SEGS:
                    emit_proj(s0 + 2)
                emit_mlp(s0)
        prev = step
    emit_md(*prev)
    emit_tail(prev[0])
    emit_mlp(prev[0])


def build_nc(meta):
    nc = bacc.Bacc("TRN2", target_bir_lowering=False, debug=False,
                   enable_asserts=True, num_devices=NCORES)
    aps = declare_tensors(nc, meta)
    with tile.TileContext(nc) as tc:
        with ExitStack() as ctx:
            build_body(ctx, tc, aps, meta)
    nc.compile()
    return nc


def in_map(core, shared):
    m = dict(dstT=core['dstT'], srcT=core['srcT'], maskb=core['maskb'])
    m.update({k: shared[k] for k in ('wb16', 'pbias')})
    return m


def assemble(xouts, inputs, meta):
    nd, doff = meta['nd'], meta['doff']
    grid, D, doffs = meta['grid'], meta['D'], meta['doffs']
    dst_h = np.asarray(inputs['dst_h'], np.float32)
    out = np.empty((int(nd.sum()), H), np.float32)
    for c_ in range(NCORES):
        x = np.asarray(xouts[c_], np.float32)  # [256, SD]
        for si in range(SEGS):
            g = grid[c_][si]
            od = doffs[si]
            seg = x[:, od:od + nd[g]].T        # [nd, 256]
            out[doff[g]:doff[g] + nd[g]] = dst_h[doff[g]:doff[g] + nd[g]] + seg
    return out


def kernel(**inputs):
    cores, shared, meta = host_prep(inputs)
    nc = build_nc(meta)
    in_maps = [in_map(cores[c_], shared) for c_ in range(NCORES)]
    res = run_bass_kernel_spmd(nc, in_maps, core_ids=list(range(NCORES)))
    xouts = [res.results[c_]["xout"] for c_ in range(NCORES)]
    return assemble(xouts, inputs, meta)


# revision 23
# speedup vs baseline: 1.0721x; 1.0721x over previous
"""Trainium2 Bass kernel for nn_CrossAttentionLayer (ragged cross-attention + MLP).

v2 design:
- 64 ragged segments -> 8 cores x 8 slots. Segments are assigned to slots by
  a small host-side optimizer so that each slot's compile-time dst width
  D[si] = max(nd) and src block count J[si] = max(ceil(ns/128)) over the 8
  cores sharing that slot are tight (the SPMD program is shared across cores).
- All matmul operands are bf16 (fp32 accumulation in PSUM). fp32r streams at
  ~2 cyc/row and disables fast-weight-load; bf16 runs 1 cyc/row.
- Activations stay channel-major [chan, tok]. V is produced directly in
  natural [tok, chan] layout using srcT as the stationary operand (no PE
  transpose, no band scatter).
- Scores: per src-block j, 4 head matmuls row-tiled (tile_position=(32h,0))
  into one PSUM tile [128, 4, 512]; exp on ScalarE in two 2-head calls with
  the src-pad mask as a per-partition bias.
- msg and den: 4-way column-tiled matmuls (tile_position=(0,32h)) accumulate
  over j; den uses a ones stationary so each head's denominator lands
  replicated across its 32 output partitions.
- 1/den via exp(-ln(den)) on ScalarE: Ln and Exp share one activation table
  set; DVE reciprocal is 8 cyc/elem and ACT Reciprocal is banned.
- The dst residual add and final transpose/unpack happen on the host.
"""
import math
import sys
from contextlib import ExitStack

import numpy as np

try:
    import concourse.bass as bass
except ImportError:
    sys.path.insert(0, "/opt/trn_rl_repo")
    import concourse.bass as bass

import concourse.tile as tile
from concourse import bacc, mybir
from concourse.bass_utils import run_bass_kernel_spmd

# The kernel alternates Exp (softmax) and Ln (reciprocal via exp(-ln(den)))
# activations. bacc's table-load pass picks the first act-table set containing
# each function, which lands Exp in "exp_and_others" and Ln in "natural_log"
# and reloads the ACT tables (~1.3us) on every switch. Both functions coexist
# only in "natural_log_exp_and_others"; hide Exp/Ln from the other sets so the
# pass is forced to pick the shared one. Dict order/size is preserved (set ids
# index into act_info.json).
_orig_get_act_tables = bacc.get_activation_tables


def _patched_get_act_tables(arch):
    tabs = _orig_get_act_tables(arch)
    A = mybir.ActivationFunctionType
    for name, fns in tabs.items():
        if name != "natural_log_exp_and_others":
            fns.discard(A.Exp)
            fns.discard(A.Ln)
    return tabs


bacc.get_activation_tables = _patched_get_act_tables

F32 = mybir.dt.float32
BF16 = mybir.dt.bfloat16

B = 64
LMAX = 512
H = 256          # h_dim
C = 128          # h_div
HEADS = 4
DH = 32
NCORES = 8
SEGS = 8         # segments (slots) per core
NPB = 6          # per-partition bias cols: bq,bk,b1a,b1b,b2a,b2b
MASK_NEG = -30000.0

# weight blob column offsets (bf16 blob)
WQ0, WQ1 = 0, 128
WK0, WK1 = 256, 384
WV0, WV1 = 512, 640
W1_0 = 768           # 3 x 256
W2_0 = 1536          # 2 x 256
ONES32 = 2048        # [128, 32] ones
BVROW = 2080         # row 0: bv (128 cols)
ONESROW = 2208       # row 0: ones (128 cols)
NW16 = 2336


def _to_bf16(x):
    import ml_dtypes
    return np.asarray(x, np.float32).astype(ml_dtypes.bfloat16)


def plan_slots(ns, nd):
    """Assign 64 segments to an 8x8 (core, slot) grid minimizing padded
    attention work. Returns grid[core][slot] = segment id, D[slot], J[slot]."""
    segs = list(range(B))
    jj = [int(math.ceil(max(int(n), 1) / 128.0)) for n in ns]

    def cost_of(order):
        # order: list of 64 seg ids; slot si <- order[8si:8si+8]
        tot = 0.0
        for si in range(SEGS):
            grp = order[8 * si:8 * si + 8]
            Dm = max(int(nd[g]) for g in grp)
            Jm = max(jj[g] for g in grp)
            tot += 4.58 * Jm * Dm + 12.7 * Dm + 560.0 * Jm
        return tot

    cand = []
    cand.append(sorted(segs, key=lambda g: int(nd[g])))
    cand.append(sorted(segs, key=lambda g: (jj[g], int(nd[g]))))
    best = min(cand, key=cost_of)
    best = list(best)
    bc = cost_of(best)
    # local search: swap segments between octiles
    improved = True
    it = 0
    while improved and it < 200:
        improved = False
        it += 1
        for a in range(B):
            for b_ in range(a + 1, B):
                if a // 8 == b_ // 8:
                    continue
                best[a], best[b_] = best[b_], best[a]
                c2 = cost_of(best)
                if c2 < bc - 1e-9:
                    bc = c2
                    improved = True
                else:
                    best[a], best[b_] = best[b_], best[a]
    # build grid: within octile si, order by core index arbitrarily. Slot
    # positions are permuted so the smallest octile leads (fast startup) and
    # the second-smallest ends the kernel (short dependency tail).
    perm = [0, 7, 6, 5, 4, 3, 2, 1]
    grid = [[0] * SEGS for _ in range(NCORES)]
    D = [0] * SEGS
    J = [0] * SEGS
    for pos in range(SEGS):
        si = perm[pos]
        grp = best[8 * si:8 * si + 8]
        D[pos] = -(-max(int(nd[g]) for g in grp) // 4) * 4  # round up to 4
        J[pos] = max(jj[g] for g in grp)
        for c_, g in enumerate(grp):
            grid[c_][pos] = g
    return grid, D, J


def host_prep(inputs):
    src_h = np.asarray(inputs['src_h'], np.float32)
    dst_h = np.asarray(inputs['dst_h'], np.float32)
    ns = np.asarray(inputs['src_num_verts']).astype(np.int64)
    nd = np.asarray(inputs['dst_num_verts']).astype(np.int64)
    soff = np.concatenate([[0], np.cumsum(ns)[:-1]])
    doff = np.concatenate([[0], np.cumsum(nd)[:-1]])

    grid, D, J = plan_slots(ns, nd)
    SD = sum(D)
    SJ = sum(J)
    SS = 128 * SJ
    doffs = np.concatenate([[0], np.cumsum(D)[:-1]])   # slot dst col offsets
    soffs = np.concatenate([[0], np.cumsum(np.array(J) * 128)[:-1]])
    joffs = np.concatenate([[0], np.cumsum(J)[:-1]])

    # channel permutation: on-chip channel chat = h*DH + d  <-> torch c = d*HEADS + h
    perm = np.empty(C, np.int64)
    for chat in range(C):
        h, d = divmod(chat, DH)
        perm[chat] = d * HEADS + h
    s = 1.0 / math.sqrt(DH)

    f32 = lambda k: np.asarray(inputs[k], np.float32)
    Wq, bq = f32('Wq'), f32('bq')
    Wk, bk = f32('Wk'), f32('bk')
    Wv, bv = f32('Wv'), f32('bv')
    Wm, bm = f32('Wm'), f32('bm')
    W1, b1 = f32('W1'), f32('b1')
    W2, b2 = f32('W2'), f32('b2')
    g1, be1, rm1, rv1 = f32('g1'), f32('be1'), f32('rm1'), f32('rv1')
    g2, be2, rm2, rv2 = f32('g2'), f32('be2'), f32('rm2'), f32('rv2')

    WqT = (Wq[perm] * s).T                      # [256, 128] (chat cols)
    bq_s = bq[perm] * s
    WkT = Wk[perm].T
    bk_r = bk[perm]
    WvT = Wv[perm].T                            # [256 cin, 128 cout(chat)]
    bv_r = bv[perm]
    Wm_p = Wm[:, perm]                          # [128, 128(chat)]
    a1 = g1 / np.sqrt(rv1 + 1e-5)
    W1_f = W1 * a1[:, None]
    b1_f = b1 * a1 + be1 - rm1 * a1
    a2 = g2 / np.sqrt(rv2 + 1e-5)
    W2_f = W2 * a2[:, None]
    b2_f = b2 * a2 + be2 - rm2 * a2
    # fold merge conv (Wm, bm) into W1's msg half
    W1m_p = W1_f[:, H:] @ Wm_p                  # [256, 128(chat)]
    b1_p = b1_f + W1_f[:, H:] @ bm
    W1T = np.concatenate([W1_f[:, :H], W1m_p], axis=1).T  # [384, 256]
    W2T = W2_f.T                                # [256, 256]

    wb16 = np.zeros((128, NW16), np.float32)
    wb16[:, WQ0:WQ0 + 128] = WqT[:128]
    wb16[:, WQ1:WQ1 + 128] = WqT[128:]
    wb16[:, WK0:WK0 + 128] = WkT[:128]
    wb16[:, WK1:WK1 + 128] = WkT[128:]
    wb16[:, WV0:WV0 + 128] = WvT[:128]
    wb16[:, WV1:WV1 + 128] = WvT[128:]
    for kk in range(3):
        wb16[:, W1_0 + kk * 256: W1_0 + (kk + 1) * 256] = W1T[kk * 128:(kk + 1) * 128]
    for kk in range(2):
        wb16[:, W2_0 + kk * 256: W2_0 + (kk + 1) * 256] = W2T[kk * 128:(kk + 1) * 128]
    wb16[:, ONES32:ONES32 + 32] = 1.0
    wb16[0, BVROW:BVROW + 128] = bv_r
    wb16[0, ONESROW:ONESROW + 128] = 1.0
    wb16 = _to_bf16(wb16)

    pbias = np.zeros((128, NPB), np.float32)
    pbias[:, 0] = bq_s
    pbias[:, 1] = bk_r
    pbias[:, 2] = b1_p[:128]
    pbias[:, 3] = b1_p[128:]
    pbias[:, 4] = b2_f[:128]
    pbias[:, 5] = b2_f[128:]

    cores = []
    for c_ in range(NCORES):
        dstT = np.zeros((2, 128, SD), np.float32)
        srcT = np.zeros((2, 128, SS), np.float32)
        maskb = np.full((128, SJ), MASK_NEG, np.float32)
        for si in range(SEGS):
            g = grid[c_][si]
            od, os_, oj = doffs[si], soffs[si], joffs[si]
            dseg = dst_h[doff[g]:doff[g] + nd[g]].T      # [256, nd]
            sseg = src_h[soff[g]:soff[g] + ns[g]].T
            dstT[0, :, od:od + nd[g]] = dseg[:128]
            dstT[1, :, od:od + nd[g]] = dseg[128:]
            srcT[0, :, os_:os_ + ns[g]] = sseg[:128]
            srcT[1, :, os_:os_ + ns[g]] = sseg[128:]
            for j in range(J[si]):
                valid = max(0, min(128, int(ns[g]) - j * 128))
                maskb[:valid, oj + j] = 0.0
        cores.append(dict(dstT=_to_bf16(dstT).reshape(2 * 128, SD),
                          srcT=_to_bf16(srcT).reshape(2 * 128, SS),
                          maskb=maskb))

    shared = dict(wb16=wb16, pbias=pbias)
    meta = dict(nd=nd, doff=doff, grid=grid, D=D, J=J, doffs=doffs,
                soffs=soffs, joffs=joffs, SD=SD, SJ=SJ, SS=SS)
    return cores, shared, meta


def declare_tensors(nc, meta):
    SD, SJ, SS = meta['SD'], meta['SJ'], meta['SS']
    aps = {}
    aps['dstT'] = nc.dram_tensor("dstT", [2 * 128, SD], BF16, kind="ExternalInput").ap()
    aps['srcT'] = nc.dram_tensor("srcT", [2 * 128, SS], BF16, kind="ExternalInput").ap()
    aps['maskb'] = nc.dram_tensor("maskb", [128, SJ], F32, kind="ExternalInput").ap()
    aps['wb16'] = nc.dram_tensor("wb16", [128, NW16], BF16, kind="ExternalInput").ap()
    aps['pbias'] = nc.dram_tensor("pbias", [128, NPB], F32, kind="ExternalInput").ap()
    aps['xout'] = nc.dram_tensor("xout", [2 * 128, SD], BF16, kind="ExternalOutput").ap()
    return aps


def build_body(ctx: ExitStack, tc: tile.TileContext, aps, meta):
    nc = tc.nc
    D, J = meta['D'], meta['J']
    SD, SJ, SS = meta['SD'], meta['SJ'], meta['SS']
    doffs, soffs, joffs = meta['doffs'], meta['soffs'], meta['joffs']

    wp = ctx.enter_context(tc.tile_pool(name="wp", bufs=1))
    inp = ctx.enter_context(tc.tile_pool(name="inp", bufs=1))
    act = ctx.enter_context(tc.tile_pool(name="act", bufs=1))
    eat = ctx.enter_context(tc.tile_pool(name="eat", bufs=1))
    out = ctx.enter_context(tc.tile_pool(name="outp", bufs=1))
    gp = ctx.enter_context(tc.tile_pool(name="gp", bufs=2, space="PSUM"))
    scp = ctx.enter_context(tc.tile_pool(name="scp", bufs=1, space="PSUM"))
    mdp = ctx.enter_context(tc.tile_pool(name="mdp", bufs=2, space="PSUM"))

    # --- inputs/weights, ordered so slot 0's operands land first ---
    dst_t = [inp.tile([128, SD], BF16, tag=f"dst{a}", name=f"dst{a}") for a in range(2)]
    src_t = [inp.tile([128, SS], BF16, tag=f"src{a}", name=f"src{a}") for a in range(2)]
    wb = wp.tile([128, NW16], BF16, tag="wb")
    pb = wp.tile([128, NPB], F32, tag="pb")
    maskb = wp.tile([128, SJ], F32, tag="maskb")
    c0d, c0s = doffs[1], soffs[1]
    g0d, g0s = doffs[4], soffs[4]
    nc.scalar.dma_start(out=wb[:, :W1_0], in_=aps['wb16'][:, :W1_0])
    for a in range(2):
        nc.scalar.dma_start(out=dst_t[a][:, :c0d], in_=aps['dstT'][a * 128:(a + 1) * 128, :c0d])
        nc.sync.dma_start(out=src_t[a][:, :c0s], in_=aps['srcT'][a * 128:(a + 1) * 128, :c0s])
    nc.sync.dma_start(out=pb[:], in_=aps['pbias'][:])
    nc.scalar.dma_start(out=maskb[:], in_=aps['maskb'][:])
    nc.sync.dma_start(out=wb[:, W1_0:], in_=aps['wb16'][:, W1_0:])
    for a in range(2):
        nc.sync.dma_start(out=dst_t[a][:, c0d:g0d], in_=aps['dstT'][a * 128:(a + 1) * 128, c0d:g0d])
        nc.sync.dma_start(out=src_t[a][:, c0s:g0s], in_=aps['srcT'][a * 128:(a + 1) * 128, c0s:g0s])
    for a in range(2):
        nc.sync.dma_start(out=dst_t[a][:, g0d:], in_=aps['dstT'][a * 128:(a + 1) * 128, g0d:])
        nc.sync.dma_start(out=src_t[a][:, g0s:], in_=aps['srcT'][a * 128:(a + 1) * 128, g0s:])

    xout_t = [out.tile([128, SD], BF16, tag=f"xo{o}", name=f"xo{o}") for o in range(2)]

    msgn_ts = {}
    # whole-core projection outputs (produced slot 0 first, rest batched)
    q_t = act.tile([128, SD], BF16, tag="q", name="q_all")
    k_t = act.tile([128, SS], BF16, tag="k", name="k_all")
    v_nat = act.tile([128, SJ, 128], BF16, tag="v", name="v_all")

    def emit_q_chunk(lo, hi):
        ps_q = gp.tile([128, 512], F32, tag="gp", name=f"psq{lo}")
        for a in range(2):
            wq_a = (WQ0, WQ1)[a]
            nc.tensor.matmul(ps_q[:, :hi - lo], wb[:, wq_a:wq_a + 128],
                             dst_t[a][:, lo:hi], start=(a == 0), stop=(a == 1))
        nc.vector.tensor_scalar_add(q_t[:, lo:hi], ps_q[:, :hi - lo], pb[:, 0:1])

    def emit_k_chunk(lo, hi):
        ps_k = gp.tile([128, 512], F32, tag="gp", name=f"psk{lo}")
        for a in range(2):
            nc.tensor.matmul(ps_k[:, :hi - lo], wb[:, (WK0, WK1)[a]:(WK0, WK1)[a] + 128],
                             src_t[a][:, lo:hi], start=(a == 0), stop=(a == 1))
        nc.vector.tensor_scalar_add(k_t[:, lo:hi], ps_k[:, :hi - lo], pb[:, 1:2])

    def emit_v_slot(si):
        # V in natural [tok, chan] layout: lhsT = srcT block, rhs = WvT half
        Js = J[si]
        os_, oj = soffs[si], joffs[si]
        ps_v = gp.tile([128, 4, 128], F32, tag="gp", name=f"psv{si}")
        for j in range(Js):
            for a in range(2):
                nc.tensor.matmul(ps_v[:, j, :], src_t[a][:, os_ + 128 * j: os_ + 128 * (j + 1)],
                                 wb[:, (WV0, WV1)[a]:(WV0, WV1)[a] + 128],
                                 start=(a == 0), stop=False)
            nc.tensor.matmul(ps_v[:, j, :], wb[0:1, ONESROW:ONESROW + 128],
                             wb[0:1, BVROW:BVROW + 128], start=False, stop=True)
        nc.vector.tensor_copy(v_nat[:, oj:oj + Js, :], ps_v[:, :Js, :])

    def chunks(lo, hi, step=512):
        return [(a, min(a + step, hi)) for a in range(lo, hi, step)]

    md_ps = {}
    e_tiles = {}

    def emit_sc_exp(si, j):
        Ds = D[si]
        od, os_, oj = doffs[si], soffs[si], joffs[si]
        if j == 0:
            md_ps[si] = (mdp.tile([128, 512], F32, tag="md", name=f"psmsg{si}"),
                         mdp.tile([128, 512], F32, tag="md", name=f"psden{si}"))
        # the scores->exp chain feeds the bottleneck engine (ScalarE): give it
        # top scheduling priority so mlp/proj matmuls never delay it
        with tc.high_priority():
            ps_sc = scp.tile([128, 4, 512], F32, tag="sc", name=f"pssc{si}_{j}")
            for h in range(HEADS):
                nc.tensor.matmul(ps_sc[:, h, :Ds],
                                 k_t[32 * h:32 * h + 32, os_ + 128 * j:os_ + 128 * (j + 1)],
                                 q_t[32 * h:32 * h + 32, od:od + Ds],
                                 start=True, stop=True, tile_position=(32 * h, 0))
            e_t = eat.tile([128, 4, 512], BF16, tag="E", name=f"E{si}_{j}", bufs=4)
            for hp in range(2):
                nc.scalar.activation(e_t[:, 2 * hp:2 * hp + 2, :Ds],
                                     ps_sc[:, 2 * hp:2 * hp + 2, :Ds],
                                     mybir.ActivationFunctionType.Exp,
                                     bias=maskb[:, oj + j: oj + j + 1])
        e_tiles[(si, j)] = e_t

    def emit_md(si, j):
        Ds, Js = D[si], J[si]
        oj = joffs[si]
        ps_msg, ps_den = md_ps[si]
        e_t = e_tiles.pop((si, j))
        for h in range(HEADS):
            nc.tensor.matmul(ps_msg[32 * h:32 * h + 32, :Ds],
                             v_nat[:, oj + j, 32 * h:32 * h + 32],
                             e_t[:, h, :Ds],
                             start=(j == 0), stop=(j == Js - 1),
                             tile_position=(0, 32 * h),
                             skip_group_check=True)
        for h in range(HEADS):
            nc.tensor.matmul(ps_den[32 * h:32 * h + 32, :Ds],
                             wb[:, ONES32:ONES32 + 32],
                             e_t[:, h, :Ds],
                             start=(j == 0), stop=(j == Js - 1),
                             tile_position=(0, 32 * h),
                             skip_group_check=True)

    def emit_tail(si):
        Ds = D[si]
        ps_msg, ps_den = md_ps.pop(si)
        ln_d = eat.tile([128, 512], F32, tag="lnd", name=f"lnd{si}", bufs=2)
        nc.scalar.activation(ln_d[:, :Ds], ps_den[:, :Ds],
                             mybir.ActivationFunctionType.Ln)
        r_t = eat.tile([128, 512], F32, tag="r", name=f"r{si}", bufs=2)
        nc.scalar.activation(r_t[:, :Ds], ln_d[:, :Ds],
                             mybir.ActivationFunctionType.Exp, scale=-1.0)
        msgn = act.tile([128, 512], BF16, tag="msgn", name=f"msgn{si}", bufs=2)
        nc.vector.tensor_mul(msgn[:, :Ds], ps_msg[:, :Ds], r_t[:, :Ds])
        msgn_ts[si] = msgn

    def emit_mlp(si):
        Ds = D[si]
        od = doffs[si]
        msgn = msgn_ts[si]
        y1 = [None, None]
        for o in range(2):
            ps_y = gp.tile([128, 512], F32, tag="gp", name=f"psy{si}_{o}")
            rhs = [dst_t[0][:, od:od + Ds], dst_t[1][:, od:od + Ds], msgn[:, :Ds]]
            for kk in range(3):
                nc.tensor.matmul(ps_y[:, :Ds],
                                 wb[:, W1_0 + 256 * kk + 128 * o: W1_0 + 256 * kk + 128 * o + 128],
                                 rhs[kk], start=(kk == 0), stop=(kk == 2))
            y1_t = act.tile([128, 512], BF16, tag=f"y1_{o}", name=f"y1_{si}_{o}", bufs=2)
            nc.vector.tensor_scalar(y1_t[:, :Ds], ps_y[:, :Ds], pb[:, 2 + o:3 + o], 0.0,
                                    op0=mybir.AluOpType.add, op1=mybir.AluOpType.max)
            y1[o] = y1_t
        for o in range(2):
            ps_z = gp.tile([128, 512], F32, tag="gp", name=f"psz{si}_{o}")
            for kk in range(2):
                nc.tensor.matmul(ps_z[:, :Ds],
                                 wb[:, W2_0 + 256 * kk + 128 * o: W2_0 + 256 * kk + 128 * o + 128],
                                 y1[kk][:, :Ds], start=(kk == 0), stop=(kk == 1))
            nc.vector.tensor_scalar_add(xout_t[o][:, od:od + Ds], ps_z[:, :Ds],
                                        pb[:, 4 + o:5 + o])

        # write back per slot
        for o in range(2):
            nc.sync.dma_start(out=aps['xout'][o * 128:(o + 1) * 128, od:od + Ds],
                              in_=xout_t[o][:, od:od + Ds])

    # explicit software pipeline over the flat (slot, j) stream: the scores/exp
    # stream runs one step ahead of the msg/den stream (so the ScalarE exp chain
    # is never starved by msg/den or mlp work), with slot-granular projections
    # two slots ahead and mlp trailing each slot's tail.
    def emit_proj(si):
        emit_q_chunk(doffs[si], doffs[si] + D[si])
        emit_k_chunk(soffs[si], soffs[si] + 128 * J[si])
        emit_v_slot(si)

    emit_proj(0)
    emit_proj(1)
    steps = [(si, j) for si in range(SEGS) for j in range(J[si])]
    prev = None
    pending = []
    for step in steps:
        emit_sc_exp(*step)
        if prev is not None:
            emit_md(*prev)
            for s0 in pending:
                emit_tail(s0)
                if s0 + 2 < SEGS:
                    emit_proj(s0 + 2)
                emit_mlp(s0)
            pending = []
            if prev[1] == J[prev[0]] - 1:
                pending.append(prev[0])
        prev = step
    emit_md(*prev)
    for s0 in pending:
        emit_tail(s0)
        emit_mlp(s0)
    emit_tail(prev[0])
    emit_mlp(prev[0])


def build_nc(meta):
    nc = bacc.Bacc("TRN2", target_bir_lowering=False, debug=False,
                   enable_asserts=True, num_devices=NCORES)
    aps = declare_tensors(nc, meta)
    with tile.TileContext(nc) as tc:
        with ExitStack() as ctx:
            build_body(ctx, tc, aps, meta)
    nc.compile()
    return nc


def in_map(core, shared):
    m = dict(dstT=core['dstT'], srcT=core['srcT'], maskb=core['maskb'])
    m.update({k: shared[k] for k in ('wb16', 'pbias')})
    return m


def assemble(xouts, inputs, meta):
    nd, doff = meta['nd'], meta['doff']
    grid, D, doffs = meta['grid'], meta['D'], meta['doffs']
    dst_h = np.asarray(inputs['dst_h'], np.float32)
    out = np.empty((int(nd.sum()), H), np.float32)
    for c_ in range(NCORES):
        x = np.asarray(xouts[c_], np.float32)  # [256, SD]
        for si in range(SEGS):
            g = grid[c_][si]
            od = doffs[si]
            seg = x[:, od:od + nd[g]].T        # [nd, 256]
            out[doff[g]:doff[g] + nd[g]] = dst_h[doff[g]:doff[g] + nd[g]] + seg
    return out


def kernel(**inputs):
    cores, shared, meta = host_prep(inputs)
    nc = build_nc(meta)
    in_maps = [in_map(cores[c_], shared) for c_ in range(NCORES)]
    res = run_bass_kernel_spmd(nc, in_maps, core_ids=list(range(NCORES)))
    xouts = [res.results[c_]["xout"] for c_ in range(NCORES)]
    return assemble(xouts, inputs, meta)
